# revision 1
# baseline (speedup 1.0000x reference)
"""Dale-constrained integrator on 8 trn2 NeuronCores.

Strategy (per sharding hint): data-parallel over batch — B=128 split as
16 per core; W / encoders / decoders replicated. The T=1024-step
recurrence runs fully on-device per core:

  s_{t+1} = mask * relu((s_t + e_t) @ M),  e_t = mask*(x0_t*enc0 + x1_t*enc1)
  o_r[t]  = dec_r . s_{t+1}
  M[k,j]  = W[j,k] * signs[k] * mask[j]

The ext term is folded into the matmul: (s+e)@M = s@M + X_t@G with
G = E@M precomputed on host (E rows = mask*enc_r). Each step on device:
  psum[c] = X_t[2,16].T @ G[2,512c] + sum_k sT[k][128,16].T @ M[k,512c]
  s' = relu(psum)            (scalar engine)
  o[t] = (dec0.s', dec1.s')  (vector engine, fused mult+accum)
  sT' = 32x32 stream-transpose + scatter back to k-tile layout (vector)
Matmuls run in float32r (full-rate PE); state/psum math in fp32.
"""
import sys
sys.path.insert(0, "/opt/trn_rl_repo")
import numpy as np
import concourse.bass as bass
import concourse.tile as tile
from concourse import bacc, mybir
from concourse.bass_utils import run_bass_kernel_spmd

N = 2048          # recurrent units
B = 16            # batch per core
NCORES = 8
T = 1024          # timesteps
NK = 16           # k-tiles of 128
CH = 512          # matmul moving chunk / one psum bank
NC_CHUNKS = 4
RING = 64         # o output ring (DMA out every RING steps)

F32 = mybir.dt.float32
F32R = mybir.dt.float32r
AF = mybir.ActivationFunctionType
OP = mybir.AluOpType

_cached_nc = None


def _build():
    nc = bacc.Bacc("TRN2", target_bir_lowering=False, debug=False)
    mmdt = F32R

    W_d = nc.dram_tensor("W", [128, NK * N], mmdt, kind="ExternalInput")
    G_d = nc.dram_tensor("G", [2, N], mmdt, kind="ExternalInput")
    s0_d = nc.dram_tensor("s0T", [128, NK * B], mmdt, kind="ExternalInput")
    xT_d = nc.dram_tensor("xT", [2, T * B], mmdt, kind="ExternalInput")
    dec_d = nc.dram_tensor("dec", [B, 2 * N], F32, kind="ExternalInput")
    o_d = nc.dram_tensor("o01", [B, 2 * T], F32, kind="ExternalOutput")

    with tile.TileContext(nc) as tc:
        with (
            tc.tile_pool(name="const", bufs=1) as cpool,
            tc.tile_pool(name="state", bufs=1) as spool,
            tc.tile_pool(name="work", bufs=2) as wpool,
            tc.tile_pool(name="oring", bufs=2) as opool,
            tc.tile_pool(name="psum", bufs=2, space="PSUM") as psum,
        ):
            W_sb = cpool.tile([128, NK * N], mmdt, tag="W")
            for kt in range(NK):
                nc.sync.dma_start(W_sb[:, kt * N:(kt + 1) * N],
                                  W_d[:, kt * N:(kt + 1) * N])
            G_sb = cpool.tile([2, N], mmdt, tag="G")
            nc.sync.dma_start(G_sb[:], G_d[:])
            dec = cpool.tile([B, 2 * N], F32, tag="dec")
            nc.sync.dma_start(dec[:], dec_d[:])

            sT_ab = [spool.tile([128, NK * B], mmdt, tag=f"sT{i}",
                                name=f"sT{i}")
                     for i in range(2)]
            nc.sync.dma_start(sT_ab[0][:], s0_d[:])
            s32 = spool.tile([32, N], F32, tag="s32")
            nc.vector.memset(s32[:], 0.0)

            xslab = opool.tile([2, RING * B], mmdt, tag="xslab",
                               name="xslab")
            nc.sync.dma_start(xslab[:], xT_d[:, 0:RING * B])

            for t in range(T):
                sin = sT_ab[t % 2]
                sout = sT_ab[(t + 1) % 2]
                oslot = t % RING
                if oslot == 0:
                    o_ring = opool.tile([B, 2, RING], F32, tag="o_ring")
                    cur_x = xslab
                    if t + RING < T:
                        xslab = opool.tile([2, RING * B], mmdt,
                                           tag="xslab", name="xslab")
                        nc.sync.dma_start(
                            xslab[:],
                            xT_d[:, (t + RING) * B:(t + 2 * RING) * B])

                for c in range(NC_CHUNKS):
                    sl = slice(c * CH, (c + 1) * CH)
                    acc = psum.tile([B, CH], F32, tag=f"ps{c}")
                    nc.tensor.matmul(acc[:],
                                     cur_x[:, oslot * B:(oslot + 1) * B],
                                     G_sb[:, sl], start=True, stop=False)
                    for r in range(NK):
                        nc.tensor.matmul(
                            acc[:], sin[:, r * B:(r + 1) * B],
                            W_sb[:, r * N + c * CH: r * N + (c + 1) * CH],
                            start=False, stop=(r == NK - 1),
                        )
                    nc.scalar.activation(s32[0:B, sl], acc[:], AF.Relu)
                    if c == 0:
                        opart = wpool.tile([B, 2, NC_CHUNKS], F32,
                                           tag="opart", name="opart")
                    for r in range(2):
                        dm = wpool.tile([B, 1], F32, tag="dots_dummy")
                        nc.vector.scalar_tensor_tensor(
                            dm[:].broadcast_to((B, CH)),
                            s32[0:B, sl], 1.0,
                            dec[:, r * N + c * CH: r * N + (c + 1) * CH],
                            op0=OP.mult, op1=OP.mult,
                            accum_out=opart[:, r, c:c + 1],
                        )
                    trq = wpool.tile([32, CH], F32, tag="trq")
                    nc.vector.transpose(trq[:], s32[:, sl])
                    for r2 in range(4):
                        dst = sout[:].rearrange(
                            "p (kt b) -> p kt b", b=B
                        )[32 * r2:32 * (r2 + 1), 4 * c:4 * c + 4, :]
                        src = trq[:].rearrange(
                            "p (tl b32) -> p tl b32", tl=4
                        )[0:32, :, 32 * r2:32 * r2 + B]
                        nc.vector.tensor_copy(dst, src)
                for r in range(2):
                    nc.vector.reduce_sum(o_ring[:, r, oslot:oslot + 1],
                                         opart[:, r, :],
                                         axis=mybir.AxisListType.X)
                if oslot == RING - 1:
                    t0 = t - RING + 1
                    for r in range(2):
                        nc.sync.dma_start(
                            o_d[:, r * T + t0: r * T + t0 + RING],
                            o_ring[:, r, :])
    nc.compile()
    return nc


def _prep_in_maps(x0, x1, enc0, enc1, dec0, dec1, W, signs, mask, state0):
    x0 = np.asarray(x0, np.float32)
    x1 = np.asarray(x1, np.float32)
    enc0 = np.asarray(enc0, np.float32)
    enc1 = np.asarray(enc1, np.float32)
    dec0 = np.asarray(dec0, np.float32)
    dec1 = np.asarray(dec1, np.float32)
    W = np.asarray(W, np.float32)
    signs = np.asarray(signs, np.float32)
    mask = np.asarray(mask, np.float32)
    state0 = np.asarray(state0, np.float32)

    # host-side constant prep (layout only + the rank-2 fold G = E @ M)
    M2 = (W * signs[None, :]).T * mask[None, :]                # [k, j]
    E = np.stack([enc0 * mask, enc1 * mask]).astype(np.float64)
    G = (E @ M2.astype(np.float64)).astype(np.float32)
    W_host = np.ascontiguousarray(
        M2.reshape(NK, 128, N).transpose(1, 0, 2).reshape(128, NK * N))
    dec = np.zeros((B, 2 * N), np.float32)
    dec[:, :N] = dec0[None, :]
    dec[:, N:] = dec1[None, :]
    s0T = np.broadcast_to(
        state0.reshape(NK, 128)[:, :, None], (NK, 128, B)
    ).transpose(1, 0, 2).reshape(128, NK * B).astype(np.float32).copy()
    shared = {"W": W_host, "G": G, "dec": dec, "s0T": s0T}

    in_maps = []
    for c in range(NCORES):
        sl = slice(c * B, (c + 1) * B)
        xT = np.empty((2, T * B), np.float32)
        xT[0] = x0[sl].T.reshape(-1)       # t-major [T*B]
        xT[1] = x1[sl].T.reshape(-1)
        in_maps.append(dict(shared, xT=xT))
    return in_maps


def kernel(x0, x1, enc0, enc1, dec0, dec1, W, signs, mask, state0):
    global _cached_nc
    in_maps = _prep_in_maps(x0, x1, enc0, enc1, dec0, dec1, W, signs,
                            mask, state0)
    if _cached_nc is None:
        _cached_nc = _build()
    res = run_bass_kernel_spmd(_cached_nc, in_maps,
                               core_ids=list(range(NCORES)))
    o0 = np.concatenate([r["o01"][:, :T] for r in res.results], axis=0)
    o1 = np.concatenate([r["o01"][:, T:] for r in res.results], axis=0)
    return (np.ascontiguousarray(o0, dtype=np.float32),
            np.ascontiguousarray(o1, dtype=np.float32))



# revision 5
# speedup vs baseline: 1.7500x; 1.7500x over previous
"""Dale-constrained integrator on 8 trn2 NeuronCores.

Data-parallel over batch (16 rows/core), W replicated. Per step on device:

  z[b, j]  = x_t @ G + sum_r sT[r].T @ M[r]     (j in 4 chunks of 512)
  s'       = relu(z); o[t-1] = dec . s_t        (dec dot via PE matmul)
  sT'      = 32x32 stream-transpose + scatter back to k-tile layout

v2 layout: all matmul operands bf16; the 4 j-chunks run CONCURRENTLY in
4 PE column groups (tile_position inferred from psum base partition
32c), so the W moving stream (the bottleneck) is 4 columns/cycle
instead of 1. One PSUM bank holds all 4 chunk outputs (partitions
32c..32c+15). Decoder dots ride the already-loaded state stationary as
N=2 matmuls accumulated over the 16 k-tiles. Relu splits across the
scalar and vector engines; scatter copies go to the idle gpsimd (Pool)
engine.
"""
import sys
sys.path.insert(0, "/opt/trn_rl_repo")
import numpy as np
from ml_dtypes import bfloat16
import concourse.bass as bass
import concourse.tile as tile
from concourse import bacc, mybir
from concourse.bass_utils import run_bass_kernel_spmd

N = 2048          # recurrent units
B = 16            # batch per core
NCORES = 8
T = 1024          # timesteps
NK = 16           # k-tiles of 128
CH = 512          # j-chunk = one col-group stream = one psum bank width
NC_CHUNKS = 4
RING = 64         # o output ring (DMA out every RING steps)

F32 = mybir.dt.float32
BF16 = mybir.dt.bfloat16
AF = mybir.ActivationFunctionType
OP = mybir.AluOpType

_cached_nc = None


def _build():
    nc = bacc.Bacc("TRN2", target_bir_lowering=False, debug=False)

    W_d = nc.dram_tensor("W", [128, NK * N], BF16, kind="ExternalInput")
    G_d = nc.dram_tensor("G", [2, N], BF16, kind="ExternalInput")
    s0_d = nc.dram_tensor("s0T", [128, NK * 32], BF16, kind="ExternalInput")
    xT_d = nc.dram_tensor("xT", [2, T * B], BF16, kind="ExternalInput")
    dec_d = nc.dram_tensor("dec", [128, NK * 2], BF16, kind="ExternalInput")
    o_d = nc.dram_tensor("o01", [B, 2 * T], F32, kind="ExternalOutput")

    with tile.TileContext(nc) as tc:
        with (
            tc.tile_pool(name="const", bufs=1) as cpool,
            tc.tile_pool(name="state", bufs=1) as spool,
            tc.tile_pool(name="work", bufs=2) as wpool,
            tc.tile_pool(name="oring", bufs=2) as opool,
            tc.tile_pool(name="psum", bufs=2, space="PSUM") as psum,
            tc.tile_pool(name="psumo", bufs=2, space="PSUM") as psumo,
        ):
            W_sb = cpool.tile([128, NK * N], BF16, tag="W")
            for kt in range(NK):
                nc.sync.dma_start(W_sb[:, kt * N:(kt + 1) * N],
                                  W_d[:, kt * N:(kt + 1) * N])
            G_sb = cpool.tile([2, N], BF16, tag="G")
            nc.sync.dma_start(G_sb[:], G_d[:])
            dec = cpool.tile([128, NK * 2], BF16, tag="dec")
            nc.sync.dma_start(dec[:], dec_d[:])

            sT_ab = [spool.tile([128, NK * 32], BF16, tag=f"sT{i}",
                                name=f"sT{i}")
                     for i in range(2)]
            nc.sync.dma_start(sT_ab[0][:], s0_d[:])
            s32 = spool.tile([32, N], BF16, tag="s32")
            nc.vector.memset(s32[:], 0.0)

            xslab = opool.tile([2, RING * B], BF16, tag="xslab",
                               name="xslab")
            nc.sync.dma_start(xslab[:], xT_d[:, 0:RING * B])

            o_ring = None
            for t in range(T + 1):       # t == T: o epilogue only
                sin = sT_ab[t % 2]
                sout = sT_ab[(t + 1) % 2]
                if t < T:
                    xslot = t % RING
                    if xslot == 0:
                        cur_x = xslab
                        if t + RING < T:
                            xslab = opool.tile([2, RING * B], BF16,
                                               tag="xslab", name="xslab")
                            nc.sync.dma_start(
                                xslab[:],
                                xT_d[:, (t + RING) * B:(t + 2 * RING) * B])
                    zps = psum.tile([128, CH], F32, tag="z")
                if t >= 1:
                    ops_ = psumo.tile([16, 2], F32, tag="o")

                if t < T:
                    xt = cur_x[:, xslot * B:(xslot + 1) * B]
                    for c in range(NC_CHUNKS):
                        nc.tensor.matmul(zps[32 * c:32 * c + B, :], xt,
                                         G_sb[:, c * CH:(c + 1) * CH],
                                         start=True, stop=False,
                                         tile_position=(0, 32 * c))
                    for r in range(NK):
                        st = sin[:, 32 * r:32 * r + B]
                        for c in range(NC_CHUNKS):
                            nc.tensor.matmul(
                                zps[32 * c:32 * c + B, :], st,
                                W_sb[:, r * N + c * CH:
                                     r * N + (c + 1) * CH],
                                start=False, stop=(r == NK - 1),
                                tile_position=(0, 32 * c))
                        if t >= 1:
                            nc.tensor.matmul(ops_[:], st,
                                             dec[:, 2 * r:2 * r + 2],
                                             start=(r == 0),
                                             stop=(r == NK - 1))
                else:
                    for r in range(NK):
                        st = sin[:, 32 * r:32 * r + B]
                        nc.tensor.matmul(ops_[:], st,
                                         dec[:, 2 * r:2 * r + 2],
                                         start=(r == 0),
                                         stop=(r == NK - 1))

                if t >= 1:
                    u = t - 1
                    uslot = u % RING
                    if uslot == 0:
                        o_ring = opool.tile([B, 2, RING], F32, tag="o_ring")
                    nc.scalar.activation(
                        o_ring[:, :, uslot:uslot + 1],
                        ops_[:].rearrange("p (r o) -> p r o", o=1),
                        AF.Copy)
                    if uslot == RING - 1:
                        u0 = u - RING + 1
                        for r in range(2):
                            nc.sync.dma_start(
                                o_d[:, r * T + u0: r * T + u0 + RING],
                                o_ring[:, r, :])

                if t < T:
                    for c in range(NC_CHUNKS):
                        src = zps[32 * c:32 * c + B, :]
                        dst = s32[0:B, c * CH:(c + 1) * CH]
                        if c % 2 == 0:
                            nc.scalar.activation(dst, src, AF.Relu)
                        else:
                            nc.vector.tensor_scalar_max(dst, src, 0.0)
                        trq = wpool.tile([32, CH], BF16, tag=f"trq{c}")
                        nc.vector.transpose(trq[:],
                                            s32[:, c * CH:(c + 1) * CH])
                        tv = trq[:].rearrange("p (a m c) -> p a m c",
                                              a=4, m=4)
                        sv = sout[:].rearrange("p (k b) -> p k b", b=32)
                        for m in range(4):
                            nc.gpsimd.tensor_copy(
                                sv[32 * m:32 * m + 32,
                                   4 * c:4 * c + 4, 0:B],
                                tv[0:32, :, m, 0:B])
    nc.compile()
    return nc


def _prep_in_maps(x0, x1, enc0, enc1, dec0, dec1, W, signs, mask, state0):
    x0 = np.asarray(x0, np.float32)
    x1 = np.asarray(x1, np.float32)
    enc0 = np.asarray(enc0, np.float32)
    enc1 = np.asarray(enc1, np.float32)
    dec0 = np.asarray(dec0, np.float32)
    dec1 = np.asarray(dec1, np.float32)
    W = np.asarray(W, np.float32)
    signs = np.asarray(signs, np.float32)
    mask = np.asarray(mask, np.float32)
    state0 = np.asarray(state0, np.float32)

    # host-side constant prep (layout only + the rank-2 fold G = E @ M)
    M2 = (W * signs[None, :]).T * mask[None, :]                # [k, j]
    E = np.stack([enc0 * mask, enc1 * mask]).astype(np.float64)
    G = (E @ M2.astype(np.float64)).astype(bfloat16)
    W_host = np.ascontiguousarray(
        M2.reshape(NK, 128, N).transpose(1, 0, 2).reshape(128, NK * N)
    ).astype(bfloat16)
    dec = np.stack([dec0, dec1], axis=-1).reshape(NK, 128, 2) \
        .transpose(1, 0, 2).reshape(128, NK * 2).astype(bfloat16)
    s0T = np.zeros((128, NK, 32), np.float32)
    s0T[:, :, :B] = state0.reshape(NK, 128).T[:, :, None]
    s0T = s0T.reshape(128, NK * 32).astype(bfloat16)
    shared = {"W": W_host, "G": G, "dec": dec, "s0T": s0T}

    in_maps = []
    for c in range(NCORES):
        sl = slice(c * B, (c + 1) * B)
        xT = np.empty((2, T * B), np.float32)
        xT[0] = x0[sl].T.reshape(-1)       # t-major [T*B]
        xT[1] = x1[sl].T.reshape(-1)
        in_maps.append(dict(shared, xT=xT.astype(bfloat16)))
    return in_maps


def kernel(x0, x1, enc0, enc1, dec0, dec1, W, signs, mask, state0):
    global _cached_nc
    in_maps = _prep_in_maps(x0, x1, enc0, enc1, dec0, dec1, W, signs,
                            mask, state0)
    if _cached_nc is None:
        _cached_nc = _build()
    res = run_bass_kernel_spmd(_cached_nc, in_maps,
                               core_ids=list(range(NCORES)))
    o0 = np.concatenate([r["o01"][:, :T] for r in res.results], axis=0)
    o1 = np.concatenate([r["o01"][:, T:] for r in res.results], axis=0)
    return (np.ascontiguousarray(o0, dtype=np.float32),
            np.ascontiguousarray(o1, dtype=np.float32))
